# revision 12
# baseline (speedup 1.0000x reference)
"""Causal multi-head self-attention on 8 Trainium2 NeuronCores.

Problem: x[2,2048,1024], 16 heads, dk=64, causal softmax, fp32 in/out.

Sharding (data + tensor parallel per the hint): core c handles batch
b = c//4 and head group g = c%4 (4 heads = 256 feature cols). wq/wk/wv
column-sharded, wo row-sharded; each core emits a fp16 [D, S] partial of
out^T for its batch; the host sums the 4 partials per batch.

Numerics (validated against the reference in a bit-faithful numpy sim):
  - q/k projections run in fp8e4 (e4m3) with the DoubleRow perf mode
    (two 128-deep k-tiles contracted per instruction): x is prescaled by
    8 and wq/wk by 256 so the 0.02-sigma weights leave fp8's subnormal
    range; the 2^22 score scale folds into the exp activation scale and
    the staircase-mask constants.
  - v projection uses an error-compensated 3-term fp8 DoubleRow split
    (x_hi@w_hi + x_lo@w_hi + x_hi@w_lo), exact to ~0.1%; the psum->sbuf
    evacuation multiplies the 1/(8*256) unscale back in.
  - everything else (scores, exp, AV, wo, output) is fp16 in/fp32 accum.
  - measured end-to-end rel err ~1.15e-2 vs the 2e-2 gate.

Per-core kernel layout (no on-device transposes anywhere):
  - scores^T tile [k=128, q<=1024] = k_h^T.T @ q_h^T, causal tiles only;
    head pairs at partition bases 0/64. The diagonal 128x128 block gets
    a staircase additive mask from one extra fp16 matmul (large-constant
    split across the two factors to stay in fp16 range at scale 2^22).
    For staircase tiles (width <= 512) both heads pack into one psum
    tile at column offsets 0/512 so a single strided exp covers both.
  - exp on ScalarE (scale fused), psum -> fp16 sbuf. AV accumulates
    v_aug.T @ e over k-tiles, trailing QK/exp by one k-tile so the PE
    never waits on exp latency. Even heads carry a ones column at col
    64 (denominator lands in psum row 64), odd heads carry it at col 0
    with dk values in cols 64:128, so the normalize multiply writes
    attnT partitions 64:128 directly - no cross-partition DMA anywhere.
  - normalization: DVE reciprocal of the two denominator rows into a
    [65, QC] fp16 tile, one K=65 PE matmul against a 0/1 selector
    broadcasts both reciprocals across partitions (rows 0:64 <- h_even,
    64:128 <- h_odd), psum -> sbuf copy, two tensor_muls -> attnT. Each
    unit's normalization is deferred into the next unit's first
    iteration so its PE matmul never head-of-line blocks on the DVE
    reciprocals.
  - projection / wo work is pumped as filler between attention steps;
    tail wo evacuations alternate DVE/ScalarE (exp queue is empty by
    then) to keep the last chunk PE-bound.
"""

import os
import sys

import numpy as np

if "/opt/trn_rl_repo" not in sys.path:
    sys.path.insert(0, "/opt/trn_rl_repo")

B, S, D, H, DK = 2, 2048, 1024, 16, 64
HPC = 4            # heads per core
GW = HPC * DK      # 256
NCORES = 8
QC = 1024          # q-chunk width
KT = 128           # k-tile
KC = D // 128      # 8 contraction chunks
XS = 8.0           # fp8 prescale on x
WWS = 256.0        # fp8 prescale on wq/wk/wv
SCALE = 1.0 / (XS * WWS) ** 2      # undoes q'*k' scale inside exp
STA_V = 46336.0                    # stair factors: product ~= -240*2^22
STB_V = -21728.0

_CACHE = {}


def _build_nc(reps=1):
    import concourse.bacc as bacc
    import concourse.tile as tile
    import concourse.bass as bass
    from concourse import mybir
    from collections import deque

    f32 = mybir.dt.float32
    f16 = mybir.dt.float16
    fp8 = mybir.dt.float8e4
    Exp = mybir.ActivationFunctionType.Exp
    PSUM = bass.MemorySpace.PSUM
    DR = mybir.MatmulPerfMode.DoubleRow

    nc = bacc.Bacc(
        "TRN2",
        target_bir_lowering=False,
        debug=False,
        enable_asserts=False,
        num_devices=NCORES,
    )

    stA_d = nc.dram_tensor("stairA", [128, 128], f16, kind="ExternalInput")
    stB_d = nc.dram_tensor("stairB", [128, 128], f16, kind="ExternalInput")
    wq8_d = nc.dram_tensor("wq8", [128, KC, GW], fp8, kind="ExternalInput")
    wk8_d = nc.dram_tensor("wk8", [128, KC, GW], fp8, kind="ExternalInput")
    x8h_d = nc.dram_tensor("x8h", [128, KC, S], fp8, kind="ExternalInput")
    x8l_d = nc.dram_tensor("x8l", [128, KC, S], fp8, kind="ExternalInput")
    wvh_d = nc.dram_tensor("wv8h", [128, KC, GW], fp8, kind="ExternalInput")
    wvl_d = nc.dram_tensor("wv8l", [128, KC, GW], fp8, kind="ExternalInput")
    wo_d = nc.dram_tensor("wo16", [128, 2, D], f16, kind="ExternalInput")
    outT_d = nc.dram_tensor("outT", [D, S], f16, kind="ExternalOutput")

    with tile.TileContext(nc) as tc:
        with (
            tc.tile_pool(name="weights", bufs=1) as wpool,
            tc.tile_pool(name="acts", bufs=1) as apool,
            tc.tile_pool(name="psmm", bufs=2, space=PSUM) as psmm,
            tc.tile_pool(name="psav", bufs=2, space=PSUM) as psav,
            tc.tile_pool(name="epool", bufs=14) as epool,
            tc.tile_pool(name="norm", bufs=2) as npool,
            tc.tile_pool(name="bcp", bufs=2) as bcpool,
            tc.tile_pool(name="avsb", bufs=4) as avpool,
            tc.tile_pool(name="outp", bufs=4) as opool,
        ):
            stA = wpool.tile([128, 128], f16, tag="stA")
            stB = wpool.tile([128, 128], f16, tag="stB")
            wq8_sb = wpool.tile([128, KC, GW], fp8, tag="wq8")
            wk8_sb = wpool.tile([128, KC, GW], fp8, tag="wk8")
            wvh_sb = wpool.tile([128, KC, GW], fp8, tag="wvh")
            wvl_sb = wpool.tile([128, KC, GW], fp8, tag="wvl")
            wo_sb = wpool.tile([128, 2, D], f16, tag="wo")
            ones65 = wpool.tile([65, 128], f16, tag="ones65")

            nc.vector.memset(ones65, 0.0)
            nc.vector.memset(ones65[0:1, 64:128], 1.0)   # h_odd recip row
            nc.vector.memset(ones65[64:65, 0:64], 1.0)   # h_even recip row

            first_rep = True
            for _rep in range(reps):  # >1 only for timing builds
                x8h_sb = apool.tile([128, KC, S], fp8, tag="x8h",
                                    name=f"x8h{_rep}")
                x8l_sb = apool.tile([128, KC, S], fp8, tag="x8l",
                                    name=f"x8l{_rep}")
                xh_view = x8h_d.ap()
                xl_view = x8l_d.ap()
                # load order gates the pipeline: wq + x(first half) feed
                # the q projection, wk the k projection, stairs the first
                # diagonal mask, wv the v projection fillers
                if first_rep:
                    nc.sync.dma_start(wq8_sb, wq8_d.ap())
                nc.sync.dma_start(x8h_sb[:, 0:4, 0:QC], xh_view[:, 0:4, 0:QC])
                if first_rep:
                    nc.sync.dma_start(wk8_sb, wk8_d.ap())
                nc.sync.dma_start(x8h_sb[:, 4:8, 0:QC], xh_view[:, 4:8, 0:QC])
                if first_rep:
                    nc.sync.dma_start(stA, stA_d.ap())
                    nc.sync.dma_start(stB, stB_d.ap())
                if first_rep:
                    nc.sync.dma_start(wvh_sb, wvh_d.ap())
                nc.sync.dma_start(x8l_sb[:, :, 0:QC], xl_view[:, :, 0:QC])
                if first_rep:
                    nc.sync.dma_start(wvl_sb, wvl_d.ap())
                nc.sync.dma_start(x8h_sb[:, :, QC:S], xh_view[:, :, QC:S])
                nc.sync.dma_start(x8l_sb[:, :, QC:S], xl_view[:, :, QC:S])
                if first_rep:
                    first_rep = False
                    nc.sync.dma_start(wo_sb, wo_d.ap())

                qT_sb = apool.tile([128, 2, S], f16, tag="qT")
                kT_sb = apool.tile([128, 2, S], f16, tag="kT")
                attnT = apool.tile([128, 2, S], f16, tag="attnT")
                # v blocks [h0, h2, h1, h3]: even heads dk at cols 0:64 +
                # ones col 64; odd heads ones col 0 + dk at cols 64:128
                v_sb = apool.tile([128, S // 128, HPC * 128], f16, tag="v")
                v4 = v_sb.rearrange("p st (hb w) -> p st hb w", w=128)
                nc.gpsimd.memset(v4[:, :, 0:2, DK:DK + 1], 1.0)
                nc.gpsimd.memset(v4[:, :, 0:2, DK + 1:128], 0.0)
                nc.gpsimd.memset(v4[:, :, 2:4, 0:1], 1.0)
                nc.gpsimd.memset(v4[:, :, 2:4, 1:DK], 0.0)
                rdens = []
                norm_ctr = [0]

                def seg2(lo=0):  # split [lo, QC) at the psum bank boundary
                    return [(lo, 512), (512, QC)] if lo < 512 else [(lo, QC)]

                def proj_qk_dst(di, m, c2, split_evac=False):
                    w_sb = (wq8_sb, wk8_sb)[di]
                    dst = (qT_sb, kT_sb)[di]
                    ps = psmm.tile([128, QC], f32, tag="mm")
                    for a, b in seg2():
                        for t in range(KC // 2):
                            nc.tensor.matmul(
                                ps[:, a:b],
                                lhsT=w_sb[:, 2 * t:2 * t + 2,
                                          128 * m:128 * (m + 1)],
                                rhs=x8h_sb[:, 2 * t:2 * t + 2,
                                           QC * c2 + a:QC * c2 + b],
                                start=(t == 0),
                                stop=(t == KC // 2 - 1),
                                perf_mode=DR,
                            )
                        if split_evac:
                            nc.vector.tensor_copy(
                                dst[:, m, QC * c2 + a:QC * c2 + b],
                                ps[:, a:b])
                    if not split_evac:
                        nc.vector.tensor_copy(
                            dst[:, m, QC * c2:QC * (c2 + 1)], ps)

                def proj_v(st):
                    # 3-term error-compensated fp8: xh@wh + xl@wh + xh@wl
                    ps = psmm.tile([128, QC], f32, tag="mm")
                    terms = []
                    for t in range(KC // 2):
                        terms.append((x8h_sb, wvh_sb, t))
                    for t in range(KC // 2):
                        terms.append((x8l_sb, wvh_sb, t))
                        terms.append((x8h_sb, wvl_sb, t))
                    n = len(terms)
                    for i, (xs, ws, t) in enumerate(terms):
                        nc.tensor.matmul(
                            ps[:, 0:GW],
                            lhsT=xs[:, 2 * t:2 * t + 2,
                                    128 * st:128 * (st + 1)],
                            rhs=ws[:, 2 * t:2 * t + 2, :],
                            start=(i == 0),
                            stop=(i == n - 1),
                            perf_mode=DR,
                        )
                    psv = ps[:, 0:GW].rearrange("p (hb w) -> p hb w", w=DK)
                    unscale = 1.0 / (XS * WWS)
                    nc.vector.tensor_scalar_mul(
                        v4[:, st, 0:2, 0:DK], psv[:, 0:2, :], unscale)
                    nc.vector.tensor_scalar_mul(
                        v4[:, st, 2:4, DK:2 * DK], psv[:, 2:4, :], unscale)

                def wo_block(dm, c2, tail_idx=-1):
                    po = psmm.tile([128, QC], f32, tag="mm")
                    for f in range(2):
                        for a, b in seg2():
                            nc.tensor.matmul(
                                po[:, a:b],
                                lhsT=wo_sb[:, f, 128 * dm:128 * (dm + 1)],
                                rhs=attnT[:, f, QC * c2 + a:QC * c2 + b],
                                start=(f == 0),
                                stop=(f == 1),
                            )
                    ob = opool.tile([128, QC], f16, tag="ob")
                    if tail_idx % 2 == 0:  # exp queue empty: use ScalarE
                        nc.scalar.copy(ob, po)
                    else:
                        nc.vector.tensor_copy(ob, po)
                    nc.sync.dma_start(
                        outT_d.ap()[128 * dm:128 * (dm + 1),
                                    QC * c2:QC * (c2 + 1)],
                        ob,
                    )

                fill = deque()

                def pump(n=1):
                    for _ in range(n):
                        if fill:
                            fill.popleft()()

                def make_norm(mi, c, avs, tail=False):
                    q0 = QC * c
                    rden = rdens[norm_ctr[0] % 2]
                    norm_ctr[0] += 1
                    asb = [avpool.tile([128, QC], f16, tag="avsb",
                                       name=f"asb{mi}{c}{hh}")
                           for hh in range(2)]

                    def emit_release():
                        # copies release the psum accumulators early; the
                        # normalize chain then runs entirely off sbuf. DVE
                        # mid-kernel (ScalarE is exp-bound); ScalarE at the
                        # tail where the exp queue is empty.
                        if tail:
                            nc.scalar.copy(asb[0], avs[0])
                            nc.scalar.copy(asb[1], avs[1])
                        else:
                            nc.vector.tensor_copy(asb[0], avs[0])
                            nc.vector.tensor_copy(asb[1], avs[1])

                    def emit_rest():
                        bc = psmm.tile([128, QC], f32, tag="mm", name="bc")
                        bs = bcpool.tile([128, QC], f16, tag="bc")
                        with nc.allow_low_precision(
                                reason="fp16 recip feeds fp16 bcast matmul"):
                            nc.vector.reciprocal(rden[64:65, :],
                                                 asb[0][64:65, :])
                            nc.vector.reciprocal(rden[0:1, :],
                                                 asb[1][0:1, :])
                        for a, b in ((0, 512), (512, QC)):
                            nc.tensor.matmul(bc[:, a:b], lhsT=ones65,
                                             rhs=rden[:, a:b],
                                             start=True, stop=True)
                            if tail:
                                nc.scalar.copy(bs[:, a:b], bc[:, a:b])
                            else:
                                nc.vector.tensor_copy(bs[:, a:b], bc[:, a:b])
                            nc.vector.tensor_mul(
                                attnT[0:DK, mi, q0 + a:q0 + b],
                                asb[0][0:DK, a:b], bs[0:DK, a:b])
                            nc.vector.tensor_mul(
                                attnT[DK:128, mi, q0 + a:q0 + b],
                                asb[1][DK:128, a:b], bs[DK:128, a:b])

                    return emit_release, emit_rest

                class Unit:
                    """One (head-pair, q-chunk) attention unit with its own
                    QK/exp stream; AV matmuls trail and are quota-drained so
                    the driver can interleave units across boundaries."""

                    def __init__(self, mi, c):
                        self.mi, self.c = mi, c
                        self.q0 = QC * c
                        self.njt = (self.q0 + QC) // KT
                        self.jA_last = self.q0 // KT + 3  # last j, vs < 512
                        self.avs = [psav.tile([128, QC], f32, tag="av",
                                              name=f"av{mi}{c}{hh}")
                                    for hh in range(2)]
                        self.pend = []
                        self.j = 0

                    def _emit_qk(self, j):
                        mi, q0 = self.mi, self.q0
                        k0 = KT * j
                        vs = max(0, k0 - q0)
                        if vs >= 512:
                            # both heads packed at column offsets 0/512;
                            # one strided exp covers both
                            w = QC - vs
                            ps = psmm.tile([128, QC], f32, tag="mm",
                                           name="ps0")
                            for hh in range(2):
                                pb = 64 * hh
                                o = 512 * hh
                                nc.tensor.matmul(
                                    ps[:, o:o + w],
                                    lhsT=kT_sb[pb:pb + DK, mi, k0:k0 + KT],
                                    rhs=qT_sb[pb:pb + DK, mi,
                                              q0 + vs:q0 + QC],
                                    start=True,
                                    stop=False,
                                )
                                nc.tensor.matmul(
                                    ps[:, o:o + KT],
                                    lhsT=stA,
                                    rhs=stB,
                                    start=False,
                                    stop=True,
                                )
                            e = epool.tile([128, QC], f16, tag="e")
                            pv = ps.rearrange("p (g z) -> p g z", z=512)
                            ev = e.rearrange("p (g z) -> p g z", z=512)
                            nc.scalar.activation(
                                ev[:, :, 0:w], pv[:, :, 0:w], Exp,
                                scale=0.125 * SCALE)
                            return vs, [e, e], [-vs, 512 - vs]
                        pss, es = [], []
                        for hh in range(2):
                            pb = 64 * hh
                            ps = psmm.tile([128, QC], f32, tag="mm",
                                           name=f"ps{hh}")
                            for a, b in seg2(vs):
                                diag_here = (k0 >= q0) and (a == vs)
                                nc.tensor.matmul(
                                    ps[:, a:b],
                                    lhsT=kT_sb[pb:pb + DK, mi, k0:k0 + KT],
                                    rhs=qT_sb[pb:pb + DK, mi,
                                              q0 + a:q0 + b],
                                    start=True,
                                    stop=not diag_here,
                                )
                                if diag_here:  # staircase causal mask
                                    nc.tensor.matmul(
                                        ps[:, vs:vs + KT],
                                        lhsT=stA,
                                        rhs=stB,
                                        start=False,
                                        stop=True,
                                    )
                            pss.append(ps)
                        for hh in range(2):
                            e = epool.tile([128, QC], f16, tag="e")
                            nc.scalar.activation(
                                e[:, vs:QC], pss[hh][:, vs:QC], Exp,
                                scale=0.125 * SCALE)
                            es.append(e)
                        return vs, es, [0, 0]

                    def _emit_av(self, j, vs, es, deltas):
                        av_ranges = []
                        if vs < 512:
                            av_ranges.append((vs, 512, j == self.jA_last))
                        av_ranges.append((max(vs, 512), QC,
                                          j == self.njt - 1))
                        for hh in range(2):
                            blk = self.mi + 2 * hh
                            dlt = deltas[hh]
                            for a, b, fin in av_ranges:
                                nc.tensor.matmul(
                                    self.avs[hh][:, a:b],
                                    lhsT=v_sb[:, j,
                                              128 * blk:128 * (blk + 1)],
                                    rhs=es[hh][:, a + dlt:b + dlt],
                                    start=(j == 0),
                                    stop=fin,
                                )

                    def step(self, av_quota, do_pump=True, hook=None):
                        new = self._emit_qk(self.j)
                        if hook is not None:
                            hook()
                        if do_pump:
                            pump(1)
                        n = 0
                        while len(self.pend) > 2 and n < av_quota:
                            self._emit_av(*self.pend.pop(0))
                            n += 1
                        self.pend.append((self.j,) + new)
                        self.j += 1

                    def drain(self):
                        while self.pend:
                            pump(1)
                            self._emit_av(*self.pend.pop(0))

                # ---- emission schedule ----
                proj_qk_dst(0, 0, 0, split_evac=True)  # q pair0 cols 0:1024
                proj_qk_dst(1, 0, 0, split_evac=True)  # k pair0
                for i in range(2):
                    r = npool.tile([65, QC], f16, tag="rden",
                                   name=f"rden{_rep}_{i}")
                    nc.vector.memset(r, 0.0)
                    rdens.append(r)
                fill.append(lambda: proj_qk_dst(0, 1, 0))
                fill.append(lambda: proj_qk_dst(1, 1, 0))
                fill.extend([lambda st=st: proj_v(st) for st in range(8)])
                fill.append(lambda: proj_qk_dst(0, 0, 1))
                fill.append(lambda: proj_qk_dst(1, 0, 1))
                fill.extend([lambda st=st: proj_v(st) for st in range(8, 14)])
                fill.append(lambda: proj_qk_dst(0, 1, 1))
                fill.append(lambda: proj_qk_dst(1, 1, 1))
                fill.extend([lambda st=st: proj_v(st)
                             for st in range(14, 16)])
                fill.extend([lambda dm=dm: wo_block(dm, 0)
                             for dm in range(8)])

                units = [Unit(0, 0), Unit(1, 0), Unit(0, 1), Unit(1, 1)]
                prev_rest = None
                for i, u in enumerate(units):
                    last = i == len(units) - 1
                    while u.j < u.njt:
                        u.step(av_quota=0 if u.j < 4 else 2,
                               hook=prev_rest if u.j == 0 else None)
                        if u.j == 1:
                            prev_rest = None
                    u.drain()
                    release, rest = make_norm(u.mi, u.c, u.avs, tail=last)
                    release()
                    prev_rest = rest
                pump(16)
                prev_rest()
                for dm in range(8):
                    wo_block(dm, 1, tail_idx=dm)

    nc.compile()
    return nc


def _get_nc():
    if "nc" not in _CACHE:
        _CACHE["nc"] = _build_nc()
    return _CACHE["nc"]


def _stairs():
    t = np.arange(128)
    stA = ((t[:, None] <= t[None, :]) * STA_V).astype(np.float16)
    stB = np.where(t[:, None] > t[None, :], STB_V, 0.0).astype(np.float16)
    return stA, stB


def _rearr_w(w):
    # [D, cols] -> [128, KC, cols]
    return np.ascontiguousarray(
        w.reshape(KC, 128, w.shape[1]).transpose(1, 0, 2))


def _make_in_maps(x, wq, wk, wv, wo):
    import ml_dtypes

    f8 = ml_dtypes.float8_e4m3
    f16 = np.float16
    stA, stB = _stairs()
    x = np.asarray(x, np.float32)
    wq = np.asarray(wq, np.float32)
    wk = np.asarray(wk, np.float32)
    wv = np.asarray(wv, np.float32)
    wo = np.asarray(wo, np.float32)

    xs, xls = [], []
    for b in range(B):
        x3 = np.ascontiguousarray(
            x[b].T.reshape(KC, 128, S).transpose(1, 0, 2)) * XS
        xh = x3.astype(f8)
        xl = (x3 - xh.astype(np.float32)).astype(f8)
        xs.append(xh)
        xls.append(xl)

    vperm = [0, 2, 1, 3]  # even heads first within the group
    in_maps = []
    for c in range(NCORES):
        b, g = divmod(c, HPC)
        cols = slice(g * GW, (g + 1) * GW)
        wvp = wv[:, cols].reshape(D, HPC, DK)[:, vperm, :].reshape(D, GW)
        wv3 = _rearr_w(wvp * WWS)
        wvh = wv3.astype(f8)
        wvl = (wv3 - wvh.astype(np.float32)).astype(f8)
        in_maps.append({
            "x8h": xs[b],
            "x8l": xls[b],
            "wq8": _rearr_w(wq[:, cols] * WWS).astype(f8),
            "wk8": _rearr_w(wk[:, cols] * WWS).astype(f8),
            "wv8h": wvh,
            "wv8l": wvl,
            "wo16": np.ascontiguousarray(
                wo[cols, :].reshape(2, 128, D).transpose(1, 0, 2)
            ).astype(f16),
            "stairA": stA,
            "stairB": stB,
        })
    return in_maps


def run(x, wq, wk, wv, wo, trace=False):
    from concourse.bass_utils import run_bass_kernel_spmd

    nc = _get_nc()
    in_maps = _make_in_maps(x, wq, wk, wv, wo)
    res = run_bass_kernel_spmd(nc, in_maps, list(range(NCORES)), trace=trace)
    acc = np.zeros((B, D, S), np.float64)
    for c in range(NCORES):
        acc[c // HPC] += res.results[c]["outT"].astype(np.float64)
    out = np.ascontiguousarray(acc.transpose(0, 2, 1).astype(np.float32))
    return out, res


def kernel(x, wq, wk, wv, wo):
    out, _ = run(x, wq, wk, wv, wo, trace=False)
    return out


# revision 13
# speedup vs baseline: 1.0621x; 1.0621x over previous
"""Causal multi-head self-attention on 8 Trainium2 NeuronCores.

Problem: x[2,2048,1024], 16 heads, dk=64, causal softmax, fp32 in/out.

Sharding (data + tensor parallel per the hint): core c handles batch
b = c//4 and head group g = c%4 (4 heads = 256 feature cols). wq/wk/wv
column-sharded, wo row-sharded; each core emits a fp16 [D, S] partial of
out^T for its batch; the host sums the 4 partials per batch.

Numerics (validated against the reference in a bit-faithful numpy sim):
  - q/k projections run in fp8e4 (e4m3) with the DoubleRow perf mode
    (two 128-deep k-tiles contracted per instruction): x is prescaled by
    8 and wq/wk by 256 so the 0.02-sigma weights leave fp8's subnormal
    range; the 2^22 score scale folds into the exp activation scale and
    the staircase-mask constants.
  - v projection uses an error-compensated 3-term fp8 DoubleRow split
    (x_hi@w_hi + x_lo@w_hi + x_hi@w_lo), exact to ~0.1%; the psum->sbuf
    evacuation multiplies the 1/(8*256) unscale back in.
  - everything else (scores, exp, AV, wo, output) is fp16 in/fp32 accum.
  - measured end-to-end rel err ~1.15e-2 vs the 2e-2 gate.

Per-core kernel layout (no on-device transposes anywhere):
  - scores^T tile [k=128, q<=1024] = k_h^T.T @ q_h^T, causal tiles only;
    head pairs at partition bases 0/64. The diagonal 128x128 block gets
    a staircase additive mask from one extra fp16 matmul (large-constant
    split across the two factors to stay in fp16 range at scale 2^22).
    For staircase tiles (width <= 512) both heads pack into one psum
    tile at column offsets 0/512 so a single strided exp covers both.
  - exp on ScalarE (scale fused), psum -> fp16 sbuf. AV accumulates
    v_aug.T @ e over k-tiles, trailing QK/exp by one k-tile so the PE
    never waits on exp latency. Even heads carry a ones column at col
    64 (denominator lands in psum row 64), odd heads carry it at col 0
    with dk values in cols 64:128, so the normalize multiply writes
    attnT partitions 64:128 directly - no cross-partition DMA anywhere.
  - normalization: DVE reciprocal of the two denominator rows into a
    [65, QC] fp16 tile, one K=65 PE matmul against a 0/1 selector
    broadcasts both reciprocals across partitions (rows 0:64 <- h_even,
    64:128 <- h_odd), psum -> sbuf copy, two tensor_muls -> attnT. Each
    unit's normalization is deferred into the next unit's first
    iteration so its PE matmul never head-of-line blocks on the DVE
    reciprocals.
  - projection / wo work is pumped as filler between attention steps;
    tail wo evacuations alternate DVE/ScalarE (exp queue is empty by
    then) to keep the last chunk PE-bound.
"""

import os
import sys

import numpy as np

if "/opt/trn_rl_repo" not in sys.path:
    sys.path.insert(0, "/opt/trn_rl_repo")

B, S, D, H, DK = 2, 2048, 1024, 16, 64
HPC = 4            # heads per core
GW = HPC * DK      # 256
NCORES = 8
QC = 1024          # q-chunk width
KT = 128           # k-tile
KC = D // 128      # 8 contraction chunks
XS = 8.0           # fp8 prescale on x
WWS = 256.0        # fp8 prescale on wq/wk/wv
SCALE = 1.0 / (XS * WWS) ** 2      # undoes q'*k' scale inside exp
STA_V = 46336.0                    # stair factors: product ~= -240*2^22
STB_V = -21728.0

_CACHE = {}


def _build_nc(reps=1):
    import concourse.bacc as bacc
    import concourse.tile as tile
    import concourse.bass as bass
    from concourse import mybir
    from collections import deque

    f32 = mybir.dt.float32
    f16 = mybir.dt.float16
    fp8 = mybir.dt.float8e4
    Exp = mybir.ActivationFunctionType.Exp
    PSUM = bass.MemorySpace.PSUM
    DR = mybir.MatmulPerfMode.DoubleRow

    nc = bacc.Bacc(
        "TRN2",
        target_bir_lowering=False,
        debug=False,
        enable_asserts=False,
        num_devices=NCORES,
    )

    stA_d = nc.dram_tensor("stairA", [128, 128], f16, kind="ExternalInput")
    stB_d = nc.dram_tensor("stairB", [128, 128], f16, kind="ExternalInput")
    wq8_d = nc.dram_tensor("wq8", [128, KC, GW], fp8, kind="ExternalInput")
    wk8_d = nc.dram_tensor("wk8", [128, KC, GW], fp8, kind="ExternalInput")
    x8h_d = nc.dram_tensor("x8h", [128, KC, S], fp8, kind="ExternalInput")
    x8l_d = nc.dram_tensor("x8l", [128, KC, S], fp8, kind="ExternalInput")
    wvh_d = nc.dram_tensor("wv8h", [128, KC, GW], fp8, kind="ExternalInput")
    wvl_d = nc.dram_tensor("wv8l", [128, KC, GW], fp8, kind="ExternalInput")
    wo_d = nc.dram_tensor("wo16", [128, 2, D], f16, kind="ExternalInput")
    outT_d = nc.dram_tensor("outT", [D, S], f16, kind="ExternalOutput")

    with tile.TileContext(nc) as tc:
        with (
            tc.tile_pool(name="weights", bufs=1) as wpool,
            tc.tile_pool(name="acts", bufs=1) as apool,
            tc.tile_pool(name="psmm", bufs=2, space=PSUM) as psmm,
            tc.tile_pool(name="psav", bufs=2, space=PSUM) as psav,
            tc.tile_pool(name="epool", bufs=14) as epool,
            tc.tile_pool(name="norm", bufs=2) as npool,
            tc.tile_pool(name="bcp", bufs=2) as bcpool,
            tc.tile_pool(name="avsb", bufs=4) as avpool,
            tc.tile_pool(name="outp", bufs=4) as opool,
        ):
            stA = wpool.tile([128, 128], f16, tag="stA")
            stB = wpool.tile([128, 128], f16, tag="stB")
            wq8_sb = wpool.tile([128, KC, GW], fp8, tag="wq8")
            wk8_sb = wpool.tile([128, KC, GW], fp8, tag="wk8")
            wvh_sb = wpool.tile([128, KC, GW], fp8, tag="wvh")
            wvl_sb = wpool.tile([128, KC, GW], fp8, tag="wvl")
            wo_sb = wpool.tile([128, 2, D], f16, tag="wo")
            ones65 = wpool.tile([65, 128], f16, tag="ones65")

            nc.vector.memset(ones65, 0.0)
            nc.vector.memset(ones65[0:1, 64:128], 1.0)   # h_odd recip row
            nc.vector.memset(ones65[64:65, 0:64], 1.0)   # h_even recip row

            first_rep = True
            for _rep in range(reps):  # >1 only for timing builds
                x8h_sb = apool.tile([128, KC, S], fp8, tag="x8h",
                                    name=f"x8h{_rep}")
                x8l_sb = apool.tile([128, KC, S], fp8, tag="x8l",
                                    name=f"x8l{_rep}")
                xh_view = x8h_d.ap()
                xl_view = x8l_d.ap()
                # load order gates the pipeline: wq + x(first half) feed
                # the q projection, wk the k projection, stairs the first
                # diagonal mask, wv the v projection fillers
                if first_rep:
                    nc.sync.dma_start(wq8_sb, wq8_d.ap())
                nc.sync.dma_start(x8h_sb[:, 0:4, 0:QC], xh_view[:, 0:4, 0:QC])
                if first_rep:
                    nc.sync.dma_start(wk8_sb, wk8_d.ap())
                nc.sync.dma_start(x8h_sb[:, 4:8, 0:QC], xh_view[:, 4:8, 0:QC])
                if first_rep:
                    nc.sync.dma_start(stA, stA_d.ap())
                    nc.sync.dma_start(stB, stB_d.ap())
                if first_rep:
                    nc.sync.dma_start(wvh_sb, wvh_d.ap())
                nc.sync.dma_start(x8l_sb[:, :, 0:QC], xl_view[:, :, 0:QC])
                if first_rep:
                    nc.sync.dma_start(wvl_sb, wvl_d.ap())
                nc.sync.dma_start(x8h_sb[:, :, QC:S], xh_view[:, :, QC:S])
                nc.sync.dma_start(x8l_sb[:, :, QC:S], xl_view[:, :, QC:S])
                if first_rep:
                    first_rep = False
                    nc.sync.dma_start(wo_sb, wo_d.ap())

                qT_sb = apool.tile([128, 2, S], f16, tag="qT")
                kT_sb = apool.tile([128, 2, S], f16, tag="kT")
                attnT = apool.tile([128, 2, S], f16, tag="attnT")
                # v blocks [h0, h2, h1, h3]: even heads dk at cols 0:64 +
                # ones col 64; odd heads ones col 0 + dk at cols 64:128
                v_sb = apool.tile([128, S // 128, HPC * 128], f16, tag="v")
                v4 = v_sb.rearrange("p st (hb w) -> p st hb w", w=128)
                nc.gpsimd.memset(v4[:, :, 0:2, DK:DK + 1], 1.0)
                nc.gpsimd.memset(v4[:, :, 0:2, DK + 1:128], 0.0)
                nc.gpsimd.memset(v4[:, :, 2:4, 0:1], 1.0)
                nc.gpsimd.memset(v4[:, :, 2:4, 1:DK], 0.0)
                rdens = []
                norm_ctr = [0]

                def seg2(lo=0):  # split [lo, QC) at the psum bank boundary
                    return [(lo, 512), (512, QC)] if lo < 512 else [(lo, QC)]

                def proj_qk_dst(di, m, c2, split_evac=False):
                    w_sb = (wq8_sb, wk8_sb)[di]
                    dst = (qT_sb, kT_sb)[di]
                    ps = psmm.tile([128, QC], f32, tag="mm")
                    for a, b in seg2():
                        for t in range(KC // 2):
                            nc.tensor.matmul(
                                ps[:, a:b],
                                lhsT=w_sb[:, 2 * t:2 * t + 2,
                                          128 * m:128 * (m + 1)],
                                rhs=x8h_sb[:, 2 * t:2 * t + 2,
                                           QC * c2 + a:QC * c2 + b],
                                start=(t == 0),
                                stop=(t == KC // 2 - 1),
                                perf_mode=DR,
                            )
                        if split_evac:
                            nc.vector.tensor_copy(
                                dst[:, m, QC * c2 + a:QC * c2 + b],
                                ps[:, a:b])
                    if not split_evac:
                        nc.vector.tensor_copy(
                            dst[:, m, QC * c2:QC * (c2 + 1)], ps)

                def proj_v(st):
                    # 3-term error-compensated fp8: xh@wh + xl@wh + xh@wl
                    ps = psmm.tile([128, QC], f32, tag="mm")
                    terms = []
                    for t in range(KC // 2):
                        terms.append((x8h_sb, wvh_sb, t))
                    for t in range(KC // 2):
                        terms.append((x8l_sb, wvh_sb, t))
                        terms.append((x8h_sb, wvl_sb, t))
                    n = len(terms)
                    for i, (xs, ws, t) in enumerate(terms):
                        nc.tensor.matmul(
                            ps[:, 0:GW],
                            lhsT=xs[:, 2 * t:2 * t + 2,
                                    128 * st:128 * (st + 1)],
                            rhs=ws[:, 2 * t:2 * t + 2, :],
                            start=(i == 0),
                            stop=(i == n - 1),
                            perf_mode=DR,
                        )
                    psv = ps[:, 0:GW].rearrange("p (hb w) -> p hb w", w=DK)
                    unscale = 1.0 / (XS * WWS)
                    nc.vector.tensor_scalar_mul(
                        v4[:, st, 0:2, 0:DK], psv[:, 0:2, :], unscale)
                    nc.vector.tensor_scalar_mul(
                        v4[:, st, 2:4, DK:2 * DK], psv[:, 2:4, :], unscale)

                def wo_block(dm, c2, tail_idx=-1):
                    po = psmm.tile([128, QC], f32, tag="mm")
                    for f in range(2):
                        for a, b in seg2():
                            nc.tensor.matmul(
                                po[:, a:b],
                                lhsT=wo_sb[:, f, 128 * dm:128 * (dm + 1)],
                                rhs=attnT[:, f, QC * c2 + a:QC * c2 + b],
                                start=(f == 0),
                                stop=(f == 1),
                            )
                    ob = opool.tile([128, QC], f16, tag="ob")
                    if tail_idx % 2 == 0:  # exp queue empty: use ScalarE
                        nc.scalar.copy(ob, po)
                    else:
                        nc.vector.tensor_copy(ob, po)
                    nc.sync.dma_start(
                        outT_d.ap()[128 * dm:128 * (dm + 1),
                                    QC * c2:QC * (c2 + 1)],
                        ob,
                    )

                fill = deque()

                def pump(n=1):
                    for _ in range(n):
                        if fill:
                            fill.popleft()()

                def make_norm(mi, c, avs, tail=False):
                    q0 = QC * c
                    rden = rdens[norm_ctr[0] % 2]
                    norm_ctr[0] += 1

                    def emit_release():
                        with nc.allow_low_precision(
                                reason="fp16 recip feeds fp16 bcast matmul"):
                            for a, b in ((0, 512), (512, QC)):
                                nc.vector.reciprocal(rden[64:65, a:b],
                                                     avs[0][64:65, a:b])
                                nc.vector.reciprocal(rden[0:1, a:b],
                                                     avs[1][0:1, a:b])

                    def emit_rest():
                        bc = psmm.tile([128, QC], f32, tag="mm", name="bc")
                        bs = bcpool.tile([128, QC], f16, tag="bc")
                        for a, b in ((0, 512), (512, QC)):
                            nc.tensor.matmul(bc[:, a:b], lhsT=ones65,
                                             rhs=rden[:, a:b],
                                             start=True, stop=True)
                            if tail:
                                nc.scalar.copy(bs[:, a:b], bc[:, a:b])
                            else:
                                nc.vector.tensor_copy(bs[:, a:b], bc[:, a:b])
                            nc.vector.tensor_mul(
                                attnT[0:DK, mi, q0 + a:q0 + b],
                                avs[0][0:DK, a:b], bs[0:DK, a:b])
                            nc.vector.tensor_mul(
                                attnT[DK:128, mi, q0 + a:q0 + b],
                                avs[1][DK:128, a:b], bs[DK:128, a:b])

                    return emit_release, emit_rest

                class Unit:
                    """One (head-pair, q-chunk) attention unit with its own
                    QK/exp stream; AV matmuls trail and are quota-drained so
                    the driver can interleave units across boundaries."""

                    def __init__(self, mi, c):
                        self.mi, self.c = mi, c
                        self.q0 = QC * c
                        self.njt = (self.q0 + QC) // KT
                        self.jA_last = self.q0 // KT + 3  # last j, vs < 512
                        self.avs = [psav.tile([128, QC], f32, tag="av",
                                              name=f"av{mi}{c}{hh}")
                                    for hh in range(2)]
                        self.pend = []
                        self.j = 0

                    def _emit_qk(self, j):
                        mi, q0 = self.mi, self.q0
                        k0 = KT * j
                        vs = max(0, k0 - q0)
                        if vs >= 512:
                            # both heads packed at column offsets 0/512;
                            # one strided exp covers both
                            w = QC - vs
                            ps = psmm.tile([128, QC], f32, tag="mm",
                                           name="ps0")
                            for hh in range(2):
                                pb = 64 * hh
                                o = 512 * hh
                                nc.tensor.matmul(
                                    ps[:, o:o + w],
                                    lhsT=kT_sb[pb:pb + DK, mi, k0:k0 + KT],
                                    rhs=qT_sb[pb:pb + DK, mi,
                                              q0 + vs:q0 + QC],
                                    start=True,
                                    stop=False,
                                )
                                nc.tensor.matmul(
                                    ps[:, o:o + KT],
                                    lhsT=stA,
                                    rhs=stB,
                                    start=False,
                                    stop=True,
                                )
                            e = epool.tile([128, QC], f16, tag="e")
                            pv = ps.rearrange("p (g z) -> p g z", z=512)
                            ev = e.rearrange("p (g z) -> p g z", z=512)
                            nc.scalar.activation(
                                ev[:, :, 0:w], pv[:, :, 0:w], Exp,
                                scale=0.125 * SCALE)
                            return vs, [e, e], [-vs, 512 - vs]
                        pss, es = [], []
                        for hh in range(2):
                            pb = 64 * hh
                            ps = psmm.tile([128, QC], f32, tag="mm",
                                           name=f"ps{hh}")
                            for a, b in seg2(vs):
                                diag_here = (k0 >= q0) and (a == vs)
                                nc.tensor.matmul(
                                    ps[:, a:b],
                                    lhsT=kT_sb[pb:pb + DK, mi, k0:k0 + KT],
                                    rhs=qT_sb[pb:pb + DK, mi,
                                              q0 + a:q0 + b],
                                    start=True,
                                    stop=not diag_here,
                                )
                                if diag_here:  # staircase causal mask
                                    nc.tensor.matmul(
                                        ps[:, vs:vs + KT],
                                        lhsT=stA,
                                        rhs=stB,
                                        start=False,
                                        stop=True,
                                    )
                            pss.append(ps)
                        for hh in range(2):
                            e = epool.tile([128, QC], f16, tag="e")
                            nc.scalar.activation(
                                e[:, vs:QC], pss[hh][:, vs:QC], Exp,
                                scale=0.125 * SCALE)
                            es.append(e)
                        return vs, es, [0, 0]

                    def _emit_av(self, j, vs, es, deltas):
                        av_ranges = []
                        if vs < 512:
                            av_ranges.append((vs, 512, j == self.jA_last))
                        av_ranges.append((max(vs, 512), QC,
                                          j == self.njt - 1))
                        for hh in range(2):
                            blk = self.mi + 2 * hh
                            dlt = deltas[hh]
                            for a, b, fin in av_ranges:
                                nc.tensor.matmul(
                                    self.avs[hh][:, a:b],
                                    lhsT=v_sb[:, j,
                                              128 * blk:128 * (blk + 1)],
                                    rhs=es[hh][:, a + dlt:b + dlt],
                                    start=(j == 0),
                                    stop=fin,
                                )

                    def step(self, av_quota, do_pump=True, hook=None):
                        new = self._emit_qk(self.j)
                        if hook is not None:
                            hook()
                        if do_pump:
                            pump(1)
                        n = 0
                        while len(self.pend) > 2 and n < av_quota:
                            self._emit_av(*self.pend.pop(0))
                            n += 1
                        self.pend.append((self.j,) + new)
                        self.j += 1

                    def drain(self):
                        while self.pend:
                            pump(1)
                            self._emit_av(*self.pend.pop(0))

                # ---- emission schedule ----
                proj_qk_dst(0, 0, 0, split_evac=True)  # q pair0 cols 0:1024
                proj_qk_dst(1, 0, 0, split_evac=True)  # k pair0
                for i in range(2):
                    r = npool.tile([65, QC], f16, tag="rden",
                                   name=f"rden{_rep}_{i}")
                    nc.vector.memset(r, 0.0)
                    rdens.append(r)
                fill.append(lambda: proj_qk_dst(0, 1, 0))
                fill.append(lambda: proj_qk_dst(1, 1, 0))
                fill.extend([lambda st=st: proj_v(st) for st in range(8)])
                fill.append(lambda: proj_qk_dst(0, 0, 1))
                fill.append(lambda: proj_qk_dst(1, 0, 1))
                fill.extend([lambda st=st: proj_v(st) for st in range(8, 14)])
                fill.append(lambda: proj_qk_dst(0, 1, 1))
                fill.append(lambda: proj_qk_dst(1, 1, 1))
                fill.extend([lambda st=st: proj_v(st)
                             for st in range(14, 16)])
                fill.extend([lambda dm=dm: wo_block(dm, 0)
                             for dm in range(8)])

                units = [Unit(0, 0), Unit(1, 0), Unit(0, 1), Unit(1, 1)]
                prev_rest = None
                for i, u in enumerate(units):
                    last = i == len(units) - 1
                    while u.j < u.njt:
                        u.step(av_quota=0 if u.j < 4 else 2,
                               hook=prev_rest if u.j == 0 else None)
                        if u.j == 1:
                            prev_rest = None
                    u.drain()
                    release, rest = make_norm(u.mi, u.c, u.avs, tail=last)
                    release()
                    prev_rest = rest
                pump(16)
                prev_rest()
                for dm in range(8):
                    wo_block(dm, 1, tail_idx=dm)

    nc.compile()
    return nc


def _get_nc():
    if "nc" not in _CACHE:
        _CACHE["nc"] = _build_nc()
    return _CACHE["nc"]


def _stairs():
    t = np.arange(128)
    stA = ((t[:, None] <= t[None, :]) * STA_V).astype(np.float16)
    stB = np.where(t[:, None] > t[None, :], STB_V, 0.0).astype(np.float16)
    return stA, stB


def _rearr_w(w):
    # [D, cols] -> [128, KC, cols]
    return np.ascontiguousarray(
        w.reshape(KC, 128, w.shape[1]).transpose(1, 0, 2))


def _make_in_maps(x, wq, wk, wv, wo):
    import ml_dtypes

    f8 = ml_dtypes.float8_e4m3
    f16 = np.float16
    stA, stB = _stairs()
    x = np.asarray(x, np.float32)
    wq = np.asarray(wq, np.float32)
    wk = np.asarray(wk, np.float32)
    wv = np.asarray(wv, np.float32)
    wo = np.asarray(wo, np.float32)

    xs, xls = [], []
    for b in range(B):
        x3 = np.ascontiguousarray(
            x[b].T.reshape(KC, 128, S).transpose(1, 0, 2)) * XS
        xh = x3.astype(f8)
        xl = (x3 - xh.astype(np.float32)).astype(f8)
        xs.append(xh)
        xls.append(xl)

    vperm = [0, 2, 1, 3]  # even heads first within the group
    in_maps = []
    for c in range(NCORES):
        b, g = divmod(c, HPC)
        cols = slice(g * GW, (g + 1) * GW)
        wvp = wv[:, cols].reshape(D, HPC, DK)[:, vperm, :].reshape(D, GW)
        wv3 = _rearr_w(wvp * WWS)
        wvh = wv3.astype(f8)
        wvl = (wv3 - wvh.astype(np.float32)).astype(f8)
        in_maps.append({
            "x8h": xs[b],
            "x8l": xls[b],
            "wq8": _rearr_w(wq[:, cols] * WWS).astype(f8),
            "wk8": _rearr_w(wk[:, cols] * WWS).astype(f8),
            "wv8h": wvh,
            "wv8l": wvl,
            "wo16": np.ascontiguousarray(
                wo[cols, :].reshape(2, 128, D).transpose(1, 0, 2)
            ).astype(f16),
            "stairA": stA,
            "stairB": stB,
        })
    return in_maps


def run(x, wq, wk, wv, wo, trace=False):
    from concourse.bass_utils import run_bass_kernel_spmd

    nc = _get_nc()
    in_maps = _make_in_maps(x, wq, wk, wv, wo)
    res = run_bass_kernel_spmd(nc, in_maps, list(range(NCORES)), trace=trace)
    acc = np.zeros((B, D, S), np.float64)
    for c in range(NCORES):
        acc[c // HPC] += res.results[c]["outT"].astype(np.float64)
    out = np.ascontiguousarray(acc.transpose(0, 2, 1).astype(np.float32))
    return out, res


def kernel(x, wq, wk, wv, wo):
    out, _ = run(x, wq, wk, wv, wo, trace=False)
    return out
